# revision 2
# baseline (speedup 1.0000x reference)
"""Trainium2 Bass kernel for nn_BasicBlock (ReActNet-style binary basic block).

Forward math (stop_gradient collapses in forward):
    s1   = sign(x + b11)                          in {-1, 0, +1}
    y1   = conv3x3(s1, sign(w1)) * (scale1*inv1) + (be1 - m1*inv1)   [BN folded]
    pre1 = y1 + x + b12
    p1   = prelu(pre1, a1)
    s2   = sign(p1 + b13 + b21) = sign(pre1 - thr)   [prelu monotone, thr host]
    y2   = conv1x1(s2, sign(w2)) * (scale2*inv2) + (be2 - m2*inv2)
    out2 = prelu(y2 + p1 + b13 + b22, a2) + b23

Convs are +-1 binary matmuls: exact in fp8e4 with fp32 PSUM.
MatmulPerfMode.DoubleRow consumes both 128-deep input-channel halves per
matmul (2x bf16 throughput).  To balance ACT vs DVE, the two input-channel
halves use different activation encodings:
  half 0: acts = sign(.) in {-1,0,1} on ACT, weights pre-halved to +-0.5
  half 1: acts = step(.) in {0,1} on DVE (tensor_scalar is_gt, 2x mode),
          weights +-1, conv pad = 0.5 (so step-encoding stays affine-exact)
  true conv = 2*psum - K, K[oc] = sum of half-1 weight signs (host const).

Epilogue (per 128-channel half, all fused single ops):
    q1 = 2A1*psum1 + x        (DVE scalar_tensor_tensor)   = pre1 - C1''
    p1 = Prelu(q1 + C1'', a1) (ACT, per-partition alpha)
    s2 = sign/step(q1 - t)    (ACT Sign half0 / DVE is_gt half1)
    q2 = 2A2*psum2 + p1       (DVE stt)
    w  = Prelu(q2 + C2x, a2)  (ACT)
    o  = w + b23              (Pool tensor_tensor, broadcast scalar)

Pipeline refinements over the first working version (each A/B-verified on
hardware via 8x-repeated programs to swamp dispatch noise):
  - q1 is drained per 512-wide hh half as soon as that half's 9-tap
    accumulation group stops, releasing the PSUM bank one half-conv earlier
    (removes the conv1(i+1) wait-for-drain stall).
  - x is loaded with ONE dma per image into a [128, 2, H, W] tile.
  - output is stored as bf16 (well inside the 2e-2 tolerance; halves store
    traffic) on the scalar-engine HWDGE ring so stores never queue behind
    x loads on the sync ring; host widens to fp32.

Sharding: data-parallel over batch, 16 images per core on 8 cores.
"""

import numpy as np

import concourse.bacc as bacc
import concourse.mybir as mybir
from concourse import tile
from concourse.bass_utils import run_bass_kernel_spmd

N_CORES = 8
B, C, H, W = 128, 256, 32, 32
PIMG = B // N_CORES  # images per core
EPS = 1e-5

_CACHE = {}


def _build_program(n_img):
    """Build the SPMD per-core Bass/Tile program (same program on all cores)."""
    f32, fp8 = mybir.dt.float32, mybir.dt.float8e4
    bf16 = mybir.dt.bfloat16
    AF = mybir.ActivationFunctionType
    ALU = mybir.AluOpType
    PM = mybir.MatmulPerfMode

    nc = bacc.Bacc("TRN2", target_bir_lowering=False, debug=False,
                   num_devices=N_CORES)

    x_d = nc.dram_tensor("x", [n_img, C, H, W], f32, kind="ExternalInput").ap()
    w1_d = nc.dram_tensor("w1t", [2, 128, 9, 2, 128], fp8,
                          kind="ExternalInput").ap()
    w2_d = nc.dram_tensor("w2t", [2, 128, 2, 128], fp8,
                          kind="ExternalInput").ap()
    cv_d = nc.dram_tensor("cv", [2, 128, 11], f32, kind="ExternalInput").ap()
    out_d = nc.dram_tensor("out", [n_img, C, H, W], bf16,
                           kind="ExternalOutput").ap()

    # cv columns: 0 b11, 1 -b11, 2 2A1, 3 C1'', 4 a1, 5 s2bias(C1''-thr),
    #             6 h2thr(thr-C1''), 7 2A2, 8 C2x, 9 a2, 10 b23
    with tile.TileContext(nc) as tc:
        with tc.tile_pool(name="wp", bufs=1) as wp, \
             tc.tile_pool(name="work", bufs=1) as work, \
             tc.tile_pool(name="ps", bufs=1, space="PSUM") as ps:

            # consts first (tiny, ONE dma) so stage_a(0) starts while weights
            # stream
            cvt = wp.tile([128, 2, 11], f32, name="cvt")
            nc.scalar.dma_start(out=cvt,
                                in_=cv_d.rearrange("t p c -> p t c"))
            cv = [cvt[:, 0], cvt[:, 1]]

            w1sb, w2sb = [], []

            def load_weights():
                # emitted after stage_a(0): x(0)+sign1(0) overlap the weight
                # stream; per-oct tiles let conv1(0) oct=0 start as soon as
                # its quarter of w1 lands
                for t in range(2):
                    w1t_ = wp.tile([128, 9, 2, 128], fp8, name=f"w1sb{t}")
                    nc.scalar.dma_start(out=w1t_, in_=w1_d[t])
                    w1sb.append(w1t_)
                for t in range(2):
                    w2t_ = wp.tile([128, 2, 128], fp8, name=f"w2sb{t}")
                    nc.scalar.dma_start(out=w2t_, in_=w2_d[t])
                    w2sb.append(w2t_)

            xts, s1ps, q1s = {}, {}, {}

            def stage_a(i):
                # load x(i) (one DMA); padded fp8 activation tile, both halves
                # on dim 1:
                #   half0 = sign(x+b11) via ACT (pad 0)
                #   half1 = (x > -b11) via DVE   (pad 0.5)
                sp = work.tile([128, 2, 34, 34], fp8, tag="s1p", bufs=3,
                               name=f"s1p_{i}")
                if i < 3:  # zero/half the pads once per round-robin buffer
                    for ct, pad in ((0, 0.0), (1, 0.5)):
                        nc.gpsimd.memset(sp[:, ct, 0, :], pad)
                        nc.gpsimd.memset(sp[:, ct, 33, :], pad)
                        nc.gpsimd.memset(sp[:, ct, 1:33, 0], pad)
                        nc.gpsimd.memset(sp[:, ct, 1:33, 33], pad)
                xt2 = work.tile([128, 2, H, W], f32, tag="xt", bufs=4,
                                name=f"xt_{i}")
                nc.sync.dma_start(
                    out=xt2,
                    in_=x_d[i].rearrange("(t p) h w -> p t h w", t=2))
                xts[i] = [xt2[:, 0], xt2[:, 1]]
                nc.scalar.activation(sp[:, 0, 1:33, 1:33], xts[i][0], AF.Sign,
                                     bias=cv[0][:, 0:1])
                nc.vector.tensor_scalar(out=sp[:, 1, 1:33, 1:33],
                                        in0=xts[i][1],
                                        scalar1=cv[1][:, 1:2], scalar2=None,
                                        op0=ALU.is_gt)
                s1ps[i] = sp

            def stage_b(i):
                # conv1(i): 36 DoubleRow matmuls; q1 = 2A1*psum + x (DVE),
                # drained per 512-wide hh half right after its 9-tap group
                # stops so the PSUM bank frees one half-conv earlier
                q1s[i] = []
                sp = s1ps[i]
                for oct in range(2):
                    p1t = ps.tile([128, 1024], f32, tag=f"ps1_{oct}",
                                  bufs=1, name=f"ps1_{oct}_{i}")
                    q1 = work.tile([128, 1024], f32, tag=f"q1_{oct}", bufs=3,
                                   name=f"q1_{oct}_{i}")
                    xflat = xts[i][oct].rearrange("p a b -> p (a b)")
                    for hh in range(2):
                        for tap in range(9):
                            kh, kw = divmod(tap, 3)
                            nc.tensor.matmul(
                                p1t[:, hh * 512:(hh + 1) * 512],
                                lhsT=w1sb[oct][:, tap],
                                rhs=sp[:, :, hh * 16 + kh:hh * 16 + kh + 16,
                                       kw:kw + 32],
                                start=(tap == 0), stop=(tap == 8),
                                perf_mode=PM.DoubleRow)
                        sl = slice(hh * 512, (hh + 1) * 512)
                        nc.vector.scalar_tensor_tensor(
                            q1[:, sl], p1t[:, sl], cv[oct][:, 2:3],
                            xflat[:, sl], op0=ALU.mult, op1=ALU.add)
                    q1s[i].append(q1)

            def stage_c(i):
                # p1 -> s2 -> conv2 -> q2 -> prelu2 -> +b23 -> store
                s2t = work.tile([128, 2, 1024], fp8, tag="s2", bufs=2,
                                name=f"s2_{i}")
                nc.scalar.activation(s2t[:, 0], q1s[i][0], AF.Sign,
                                     bias=cv[0][:, 5:6])
                nc.vector.tensor_scalar(out=s2t[:, 1], in0=q1s[i][1],
                                        scalar1=cv[1][:, 6:7], scalar2=None,
                                        op0=ALU.is_gt)
                p1s = []
                for ct in range(2):
                    p1 = work.tile([128, 1024], f32, tag=f"p1_{ct}", bufs=2,
                                   name=f"p1_{ct}_{i}")
                    nc.scalar.activation(p1, q1s[i][ct], AF.Prelu,
                                         bias=cv[ct][:, 3:4],
                                         alpha=cv[ct][:, 4:5])
                    p1s.append(p1)
                split = 2 if i >= n_img - 2 else 1
                for oct in range(2):
                    p2t = ps.tile([128, 1024], f32, tag=f"ps2_{oct}", bufs=1,
                                  name=f"ps2_{oct}_{i}")
                    q2 = work.tile([128, 1024], f32, tag=f"q2_{oct}", bufs=2,
                                   name=f"q2_{oct}_{i}")
                    wt = work.tile([128, 1024], f32, tag=f"wt_{oct}", bufs=2,
                                   name=f"wt_{oct}_{i}")
                    ot = work.tile([128, 1024], bf16, tag=f"ot_{oct}", bufs=3,
                                   name=f"ot_{oct}_{i}")
                    for hh in range(2):
                        nc.tensor.matmul(
                            p2t[:, hh * 512:(hh + 1) * 512],
                            lhsT=w2sb[oct],
                            rhs=s2t[:, :, hh * 512:(hh + 1) * 512],
                            start=True, stop=True,
                            perf_mode=PM.DoubleRow)
                    for ss in range(split):
                        sl = slice(ss * 1024 // split, (ss + 1) * 1024 // split)
                        rsl = slice(ss * H // split, (ss + 1) * H // split)
                        nc.vector.scalar_tensor_tensor(
                            q2[:, sl], p2t[:, sl], cv[oct][:, 7:8],
                            p1s[oct][:, sl], op0=ALU.mult, op1=ALU.add)
                        nc.scalar.activation(wt[:, sl], q2[:, sl], AF.Prelu,
                                             bias=cv[oct][:, 8:9],
                                             alpha=cv[oct][:, 9:10])
                        nc.gpsimd.tensor_tensor(
                            out=ot[:, sl], in0=wt[:, sl],
                            in1=cv[oct][:, 10:11].broadcast_to(
                                [128, 1024 // split]),
                            op=ALU.add)
                        nc.scalar.dma_start(
                            out=out_d[i, oct * 128:(oct + 1) * 128, rsl, :],
                            in_=ot[:, sl].rearrange("p (a b) -> p a b",
                                                    a=H // split))

            # PE warm-up: dummy matmuls on a zeroed tile run during the
            # startup DMA wait so conv1(0) starts at full HAM clock rate
            warm = work.tile([128, 512], bf16, name="warm")
            nc.gpsimd.memset(warm, 0.0)
            wps = ps.tile([128, 1024], f32, tag="ps1_0", bufs=1, name="wps")
            for r in range(18):
                nc.tensor.matmul(wps[:, 0:512], lhsT=warm[:, 0:128], rhs=warm,
                                 start=(r == 0), stop=(r == 17))

            for it in range(n_img + 2):
                if it < n_img:
                    stage_a(it)
                if it == 0:
                    load_weights()
                if 1 <= it <= n_img:
                    stage_b(it - 1)
                if 2 <= it:
                    stage_c(it - 2)

    nc.compile()
    return nc


def _prep_host(inputs):
    """Host-side O(C^2) weight/constant preprocessing (numpy)."""
    f = lambda k: np.asarray(inputs[k], dtype=np.float32)
    w1, w2 = f("w1"), f("w2")
    b11, b12, b13 = f("b11"), f("b12"), f("b13")
    b21, b22, b23 = f("b21"), f("b22"), f("b23")
    a1, a2 = f("a1"), f("a2")
    g1, be1, m1, v1 = f("g1m"), f("be1m"), f("m1m"), f("v1m")
    g2, be2, m2, v2 = f("g2m"), f("be2m"), f("m2m"), f("v2m")

    fp8 = mybir.dt.np(mybir.dt.float8e4)

    scale1 = np.abs(w1).mean(axis=(1, 2, 3), dtype=np.float64).astype(np.float32)
    scale2 = np.abs(w2).mean(axis=(1, 2, 3), dtype=np.float64).astype(np.float32)

    # sign(w1): [oc, ic, kh, kw] -> [oct, ic_lo, tap, ict, oc_lo];
    # input-channel half 0 (ict=0) pre-halved (its acts are +-1 sign form),
    # half 1 stays +-1 (acts are {0,1} step form, corrected by K1).
    sgn1 = np.sign(w1)
    K1 = sgn1[:, 128:].sum(axis=(1, 2, 3))  # [oc]
    t1 = sgn1.reshape(2, 128, 2, 128, 3, 3).transpose(0, 3, 4, 5, 2, 1)
    t1 = np.ascontiguousarray(t1).reshape(2, 128, 9, 2, 128)
    t1[:, :, :, 0, :] *= 0.5
    w1t = t1.astype(fp8)
    # sign(w2): [oc, ic] -> [oct, ic_lo, ict, oc_lo]
    sgn2 = np.sign(w2[:, :, 0, 0])
    K2 = sgn2[:, 128:].sum(axis=1)  # [oc]
    t2 = sgn2.reshape(2, 128, 2, 128).transpose(0, 3, 2, 1).copy()
    t2[:, :, 0, :] *= 0.5
    w2t = np.ascontiguousarray(t2).astype(fp8)

    inv1 = g1 / np.sqrt(v1 + EPS)
    inv2 = g2 / np.sqrt(v2 + EPS)
    A1 = scale1 * inv1
    C1 = be1 - m1 * inv1 + b12
    A2 = scale2 * inv2
    C2p = be2 - m2 * inv2 + b22 + b13
    C1pp = C1 - A1 * K1
    C2x = C2p - A2 * K2
    sb = b13 + b21
    thr = np.where(sb > 0, -sb / a1, -sb)  # s2 = sign(pre1 - thr)
    cv = np.stack([
        b11, -b11, 2.0 * A1, C1pp, a1, C1pp - thr,
        thr - C1pp, 2.0 * A2, C2x, a2, b23,
    ], axis=-1).astype(np.float32).reshape(2, 128, 11)
    return w1t, w2t, np.ascontiguousarray(cv)


def _make_runner(nc):
    """Persistent jitted 8-core executor (compiles once, reusable across
    kernel() calls). Mirrors bass2jax.run_bass_via_pjrt's multi-core path."""
    import jax
    from jax.experimental.shard_map import shard_map
    from jax.sharding import Mesh, PartitionSpec
    from concourse.bass2jax import (install_neuronx_cc_hook, _bass_exec_p,
                                    partition_id_tensor)

    install_neuronx_cc_hook()
    pname = nc.partition_id_tensor.name if nc.partition_id_tensor else None
    in_names, out_names, out_avals, zero_outs = [], [], [], []
    for alloc in nc.m.functions[0].allocations:
        if not isinstance(alloc, mybir.MemoryLocationSet):
            continue
        name = alloc.memorylocations[0].name
        if alloc.kind == "ExternalInput":
            if name != pname:
                in_names.append(name)
        elif alloc.kind == "ExternalOutput":
            out_names.append(name)
            shape = tuple(alloc.tensor_shape)
            dtype = mybir.dt.np(alloc.dtype)
            out_avals.append(jax.core.ShapedArray(shape, dtype))
            zero_outs.append(np.zeros(shape, dtype))
    all_names = in_names + out_names + ([pname] if pname else [])

    def _body(*args):
        operands = list(args)
        if pname is not None:
            operands.append(partition_id_tensor())
        return tuple(_bass_exec_p.bind(
            *operands, out_avals=tuple(out_avals), in_names=tuple(all_names),
            out_names=tuple(out_names), lowering_input_output_aliases=(),
            sim_require_finite=True, sim_require_nnan=True, nc=nc))

    devices = jax.devices()[:N_CORES]
    assert len(devices) == N_CORES
    mesh = Mesh(np.asarray(devices), ("core",))
    spec = PartitionSpec("core")
    n_args = len(in_names) + len(out_names)
    jitted = jax.jit(
        shard_map(_body, mesh=mesh, in_specs=(spec,) * n_args,
                  out_specs=(spec,) * len(out_names), check_rep=False),
        keep_unused=True,
    )

    def run(per_core_in):
        concat_in = [np.concatenate([m[nm] for m in per_core_in], axis=0)
                     for nm in in_names]
        concat_zeros = [np.zeros((N_CORES * z.shape[0], *z.shape[1:]), z.dtype)
                        for z in zero_outs]
        outs = jitted(*concat_in, *concat_zeros)
        oix = out_names.index("out")
        return np.asarray(outs[oix])  # [N_CORES*PIMG, C, H, W] bf16

    return run


def kernel(**inputs):
    x = np.ascontiguousarray(np.asarray(inputs["x"], dtype=np.float32))
    w1t, w2t, cv = _prep_host(inputs)

    if "nc" not in _CACHE:
        _CACHE["nc"] = _build_program(PIMG)
    nc = _CACHE["nc"]

    in_maps = [{
        "x": x[c * PIMG:(c + 1) * PIMG],
        "w1t": w1t,
        "w2t": w2t,
        "cv": cv,
    } for c in range(N_CORES)]

    try:
        if "runner" not in _CACHE:
            _CACHE["runner"] = _make_runner(nc)
        out = _CACHE["runner"](in_maps)
    except Exception:
        _CACHE.pop("runner", None)
        res = run_bass_kernel_spmd(nc, in_maps, core_ids=list(range(N_CORES)))
        out = np.concatenate([r["out"] for r in res.results], axis=0)
    return np.asarray(out).astype(np.float32)
